# revision 13
# baseline (speedup 1.0000x reference)
"""Multi-head causal attention (B=2, S=2048, E=1024, H=16, D=64) on 8 trn2 cores.

Sharding (Megatron-style, per hint): data-parallel over batch (2) x
tensor-parallel over heads (4 groups of 4 heads / 256 features).
Core c: batch c//4, head-group c%4.

Per-core device program (SPMD, identical on all cores), pipelined over 4
query chunks of 512, all matmuls in bf16:
  A(ic). DVE-convert x rows to bf16, PE-transpose -> xT for the chunk
  B(ic). qT/kT projections in [n, s] layout; v in natural [s, n] layout
  C(ic). causal attention in transposed-score layout, processed in HEAD
       PAIRS so the two K=64 score matmuls sit in PE row-halves 0:64 /
       64:128 and overlap (PE quadrant packing):
       sT[j,i] = kT_h . qT_h, p = exp(s/8) on ScalarE (bf16),
       causal mask via multiplicative 0/1 tile on diagonal blocks,
       ctxT[d,i] accumulated with v-augmented-with-ones stationary ->
       row 64 of psum = softmax denominator; normalize with
       1/L = exp(-ln L) on ScalarE tables + partition_broadcast + multiply
       (keeps the slow elementwise reciprocal off the DVE queue)
  D(ic). AllGather the normalized ctxT chunk across the 4-core TP group
       (chunks 0-2: one collective; chunk 3: two row-half collectives,
       one per head pair, so the tail collective hides behind pair 1)
  E(ic). out[:, g*256:(g+1)*256] = ctxT_full.T @ Wo[:, slice] + bo[slice]
       (E(ic) is emitted as fillers inside C(ic+1)/C(ic+2) so the
       collectives hide behind compute)
Host only slices inputs and concatenates the 8 disjoint output slices.
"""

import contextlib

import ml_dtypes
import numpy as np

import concourse.mybir as mybir
import concourse.tile as tile
from concourse import bacc
from concourse.bass_utils import run_bass_kernel_spmd

F32 = mybir.dt.float32
BF16 = mybir.dt.bfloat16

B, S, E, H, D = 2, 2048, 1024, 16, 64
N_CORES = 8
TP = 4                 # tensor-parallel degree (head groups per batch)
NSL = E // TP          # 256 features per core
HLOC = H // TP         # 4 heads per core
KT = E // 128          # 8 contraction tiles
IT = S // 128          # 16 sequence tiles
ICH = S // 512         # 4 sequence chunks of 512
SCALE = 1.0 / np.sqrt(D)

REPLICA_GROUPS = [[0, 1, 2, 3], [4, 5, 6, 7]]

_cache: dict = {}


def _emit(nc, tc, prm):
    x, wq, bq, wk, bk, wv, bv, wo, bo, ident, tri, out = prm

    with contextlib.ExitStack() as stack:
        ent = stack.enter_context
        const = ent(tc.tile_pool(name="const", bufs=1))
        wstage = ent(tc.tile_pool(name="wstage", bufs=8))
        wpool = ent(tc.tile_pool(name="wpool", bufs=1))
        xrow_p = ent(tc.tile_pool(name="xrow", bufs=2))
        xt_p = ent(tc.tile_pool(name="xt", bufs=2))
        qkv_p = ent(tc.tile_pool(name="qkv", bufs=1))
        psum_t = ent(tc.tile_pool(name="psum_t", bufs=2, space="PSUM"))
        psum_mm = ent(tc.tile_pool(name="psum_mm", bufs=2, space="PSUM"))
        psum_s = ent(tc.tile_pool(name="psum_s", bufs=2, space="PSUM"))
        psum_c = ent(tc.tile_pool(name="psum_c", bufs=2, space="PSUM"))
        pwork = ent(tc.tile_pool(name="pwork", bufs=6))
        norm_p = ent(tc.tile_pool(name="norm", bufs=2))
        ctxt_p = ent(tc.tile_pool(name="ctxt", bufs=1))
        ctxf_p = ent(tc.tile_pool(name="ctxf", bufs=2))
        osb_p = ent(tc.tile_pool(name="osb", bufs=2))
        dram = ent(tc.tile_pool(name="dram", bufs=1, space="DRAM"))

        # ---- constants ----
        ident_sb = const.tile([128, 128], BF16)
        nc.sync.dma_start(out=ident_sb[:], in_=ident[:])
        tri_sb = const.tile([128, 128], BF16)
        nc.sync.dma_start(out=tri_sb[:], in_=tri[:])
        ones_sb = const.tile([1, D], BF16)
        nc.vector.memset(ones_sb[:], 1.0)

        # ---- persistent activations ----
        qt_sb = qkv_p.tile([128, 2, S], BF16)
        kt_sb = qkv_p.tile([128, 2, S], BF16)
        v_sb = qkv_p.tile([128, IT, HLOC, D + 1], BF16)
        nc.vector.memset(v_sb[:, :, :, D:D + 1], 1.0)
        ctxt_sb = ctxt_p.tile([128, 2, S], BF16)

        # ---- stage A: x rows -> bf16 -> PE transpose ----
        def stage_a_alloc(ic):
            xt_sb = xt_p.tile([128, KT, 512], BF16, tag="xt")
            xrbs = []
            for k4, it in enumerate(range(4 * ic, 4 * ic + 4)):
                xr = xrow_p.tile([128, E], F32, tag="xr", bufs=8)
                xrb = xrow_p.tile([128, E], BF16, tag="xrb", bufs=8)
                for h in range(2):
                    sl = slice(h * (E // 2), (h + 1) * (E // 2))
                    nc.sync.dma_start(
                        out=xr[:, sl], in_=x[it * 128:(it + 1) * 128, sl])
                    nc.vector.tensor_copy(xrb[:, sl], xr[:, sl])
                xrbs.append(xrb)
            return xt_sb, xrbs

        def stage_a_part(xt_sb, xrbs, kt):
            pt = psum_t.tile([128, 512], BF16, tag="pt")
            for k4 in range(4):
                nc.tensor.transpose(
                    pt[:, k4 * 128:(k4 + 1) * 128],
                    xrbs[k4][:, kt * 128:(kt + 1) * 128],
                    ident_sb[:])
            nc.vector.tensor_copy(xt_sb[:, kt, :], pt[:])

        xt_cur, xrbs_cur = stage_a_alloc(0)
        for part in range(KT):
            stage_a_part(xt_cur, xrbs_cur, part)

        # ---- weights: per-kt DMA (gpsimd SWDGE queue) + DVE bf16 convert ----
        def stage_w(w_sb, w_dr):
            w_r = w_dr.rearrange("(t p) n -> p t n", p=128)
            for kt in range(KT):
                wst = wstage.tile([128, NSL], F32, tag="wst", bufs=8)
                nc.gpsimd.dma_start(out=wst[:], in_=w_r[:, kt, :])
                nc.vector.tensor_copy(w_sb[:, kt, :], wst[:])

        wq_sb = wpool.tile([128, KT, NSL], BF16)
        wk_sb = wpool.tile([128, KT, NSL], BF16)
        wv_sb = wpool.tile([128, KT, NSL], BF16)
        wo_sb = wpool.tile([128, KT, NSL], BF16)
        stage_w(wq_sb, wq)
        stage_w(wk_sb, wk)
        stage_w(wv_sb, wv)
        bq_sb = wpool.tile([128, 2], F32)
        bk_sb = wpool.tile([128, 2], F32)
        for b_sb, b_dr in ((bq_sb, bq), (bk_sb, bk)):
            nc.sync.dma_start(out=b_sb[:], in_=b_dr.rearrange("(t p) -> p t", p=128))
        bv_row = wpool.tile([1, NSL], F32)
        nc.sync.dma_start(out=bv_row[:], in_=bv[None, :])
        bvb = wpool.tile([128, NSL], F32)
        nc.gpsimd.partition_broadcast(out_ap=bvb[:], in_ap=bv_row[:])

        # ---- stage B: projections ----
        def stage_b(ic, xt_sb):
            for (w_sb, b_sb, o_sb) in ((wq_sb, bq_sb, qt_sb),
                                       (wk_sb, bk_sb, kt_sb)):
                osl = slice(ic * 512, (ic + 1) * 512)
                for nt in range(2):
                    pm = psum_mm.tile([128, 512], F32, tag="pqk")
                    for kt in range(KT):
                        nc.tensor.matmul(
                            pm[:],
                            w_sb[:, kt, nt * 128:(nt + 1) * 128],
                            xt_sb[:, kt, :],
                            start=(kt == 0), stop=(kt == KT - 1),
                        )
                    nc.vector.tensor_scalar_add(
                        out=o_sb[:, nt, osl],
                        in0=pm[:], scalar1=b_sb[:, nt:nt + 1])
            for k4, it in enumerate(range(4 * ic, 4 * ic + 4)):
                pv = psum_mm.tile([128, NSL], F32, tag="pqk")
                for kt in range(KT):
                    nc.tensor.matmul(
                        pv[:],
                        xt_sb[:, kt, k4 * 128:(k4 + 1) * 128],
                        wv_sb[:, kt, :],
                        start=(kt == 0), stop=(kt == KT - 1),
                    )
                nc.vector.tensor_add(
                    out=v_sb[:, it, :, 0:D],
                    in0=pv[:].rearrange("p (h d) -> p h d", d=D),
                    in1=bvb[:].rearrange("p (h d) -> p h d", d=D))

        # DRAM bounce buffers for the chunked allgather. Chunks 0+1 share one
        # buffer and one merged collective (per-op fixed cost ~8-10us, so
        # fewer, bigger gathers win); chunk 2 is alone; chunk 3 goes as two
        # row-half gathers so the tail collective is small.
        cc_in01 = dram.tile([NSL, 2 * 512], BF16)
        cc_out01 = dram.tile([TP, NSL, 2 * 512], BF16)
        cc_in2 = dram.tile([NSL, 512], BF16)
        cc_out2 = dram.tile([TP, NSL, 512], BF16)
        cc_in3 = dram.tile([NSL, 512], BF16)
        cc3_out = [dram.tile([TP, 128, 512], BF16, name=f"cc3_out{h}")
                   for h in range(2)]

        def cc_in_ap(ic, nt):
            rsl = slice(nt * 128, (nt + 1) * 128)
            if ic <= 1:
                return cc_in01[rsl, ic * 512:(ic + 1) * 512]
            if ic == 2:
                return cc_in2[rsl, :]
            return cc_in3[rsl, :]

        def stage_d01():
            nc.gpsimd.collective_compute(
                "AllGather", mybir.AluOpType.bypass,
                replica_groups=REPLICA_GROUPS,
                ins=[cc_in01.opt()], outs=[cc_out01.opt()],
            )

        def stage_d2():
            nc.gpsimd.collective_compute(
                "AllGather", mybir.AluOpType.bypass,
                replica_groups=REPLICA_GROUPS,
                ins=[cc_in2.opt()], outs=[cc_out2.opt()],
            )

        def stage_d3_half(nt):
            nc.gpsimd.collective_compute(
                "AllGather", mybir.AluOpType.bypass,
                replica_groups=REPLICA_GROUPS,
                ins=[cc_in3[nt * 128:(nt + 1) * 128, :].opt()],
                outs=[cc3_out[nt].opt()],
            )

        # ---- stage C: causal attention, head pairs ----
        def stage_c(ic, fillers):
            i0 = ic * 512
            njt = 4 * (ic + 1)

            def emit_s(h, jt):
                # diagonal j-tiles only need columns i_local >= 128*dt_
                nt = h // 2
                base = (h % 2) * D
                dt_ = jt - 4 * ic
                c0 = max(dt_, 0) * 128
                ps = psum_s.tile([128, 512], F32, tag="ps")
                nc.tensor.matmul(
                    ps[:, c0:],
                    kt_sb[base:base + D, nt, jt * 128:(jt + 1) * 128],
                    qt_sb[base:base + D, nt, i0 + c0:i0 + 512],
                    start=True, stop=True,
                )
                pw = pwork.tile([128, 512], BF16, tag="pw")
                nc.scalar.activation(
                    out=pw[:, c0:], in_=ps[:, c0:],
                    func=mybir.ActivationFunctionType.Exp, scale=float(SCALE))
                if dt_ >= 0:
                    nc.vector.tensor_mul(
                        pw[:, c0:c0 + 128], pw[:, c0:c0 + 128], tri_sb[:])
                return pw

            def emit_ctx(h, jt, pc, pw):
                c0 = max(jt - 4 * ic, 0) * 128
                nc.tensor.matmul(
                    pc[:, c0:],
                    v_sb[:, jt, h, :],
                    pw[:, c0:],
                    start=(jt == 0), stop=(jt == njt - 1),
                )

            for hp in range(2):
                h0, h1 = 2 * hp, 2 * hp + 1
                nt = hp
                pcs = [psum_c.tile([D + 1, 512], F32, tag="pc", name=f"pc{i}")
                       for i in range(2)]
                # score matmuls for the pair are adjacent: K=64 stationaries
                # at PE row-halves 0:64 / 64:128 execute concurrently
                pws = [emit_s(h0, 0), emit_s(h1, 0)]
                for jt in range(1, njt):
                    nxt = [emit_s(h0, jt), emit_s(h1, jt)]
                    emit_ctx(h0, jt - 1, pcs[0], pws[0])
                    emit_ctx(h1, jt - 1, pcs[1], pws[1])
                    pws = nxt
                emit_ctx(h0, njt - 1, pcs[0], pws[0])
                emit_ctx(h1, njt - 1, pcs[1], pws[1])
                for idx, h in enumerate((h0, h1)):
                    base = (h % 2) * D
                    # 1/L = exp(-ln L) via two ACT table ops
                    lnrow = norm_p.tile([1, 512], F32, tag="lnrow")
                    nc.scalar.activation(
                        out=lnrow[:], in_=pcs[idx][D:D + 1, :],
                        func=mybir.ActivationFunctionType.Ln)
                    lrow = norm_p.tile([1, 512], F32, tag="lrow")
                    nc.scalar.activation(
                        out=lrow[:], in_=lnrow[:],
                        func=mybir.ActivationFunctionType.Exp, scale=-1.0)
                    lb = norm_p.tile([D, 512], F32, tag="lb")
                    nc.gpsimd.partition_broadcast(out_ap=lb[:], in_ap=lrow[:])
                    nc.vector.tensor_mul(
                        ctxt_sb[base:base + D, nt, i0:i0 + 512],
                        pcs[idx][0:D, :], lb[:])
                nc.sync.dma_start(
                    out=cc_in_ap(ic, nt),
                    in_=ctxt_sb[:, nt, i0:i0 + 512])
                if ic == ICH - 1:
                    stage_d3_half(nt)
                    stage_e3_load(nt)
                elif nt == 1:
                    if ic == 1:
                        stage_d01()
                        stage_e01_load()
                    elif ic == 2:
                        stage_d2()
                        stage_e2_load()
                for f in fillers.get(hp, []):
                    f()

        # ---- stage E: out-projection ----
        # e-loads go on the gpsimd queue: they block waiting for their
        # gather, and gpsimd is the one queue where that's harmless (the CC
        # stream is serial, so the next collective couldn't start earlier
        # anyway). Keeping them off scalar/sync keeps exp and x-loads flowing.
        efs = {}

        def stage_e01_load():
            ctxf_sb = ctxf_p.tile([128, KT, 2 * 512], BF16, tag="ctxf01",
                                  bufs=1)
            nc.gpsimd.dma_start(
                out=ctxf_sb[:],
                in_=cc_out01.rearrange("g (t p) i -> p (g t) i", p=128))
            efs["01"] = ctxf_sb

        def stage_e2_load():
            ctxf_sb = ctxf_p.tile([128, KT, 512], BF16, tag="ctxf2", bufs=1)
            nc.gpsimd.dma_start(
                out=ctxf_sb[:],
                in_=cc_out2.rearrange("g (t p) i -> p (g t) i", p=128))
            efs["2"] = ctxf_sb

        def stage_e_part(ic, k):
            ctxf_sb = efs["01"] if ic <= 1 else efs["2"]
            off = ic * 512 if ic <= 1 else 0
            it = 4 * ic + k
            po = psum_mm.tile([128, NSL], F32, tag="pqk")
            for ct in range(KT):
                nc.tensor.matmul(
                    po[:],
                    ctxf_sb[:, ct, off + k * 128:off + (k + 1) * 128],
                    wo_sb[:, ct, :],
                    start=(ct == 0), stop=(ct == KT - 1),
                )
            ot = osb_p.tile([128, NSL], F32, tag="ot")
            nc.vector.tensor_add(out=ot[:], in0=po[:], in1=bob[:])
            nc.sync.dma_start(
                out=out[it * 128:(it + 1) * 128, :], in_=ot[:])

        # last chunk: two row-half gathers, accumulate evens then odds so the
        # second collective hides behind the even-half matmuls
        cf3 = [None, None]

        def stage_e3_load(nt):
            cf = ctxf_p.tile([128, TP, 512], BF16, tag=f"cf3_{nt}", bufs=1)
            nc.gpsimd.dma_start(
                out=cf[:], in_=cc3_out[nt].rearrange("g p i -> p g i"))
            cf3[nt] = cf

        def stage_e3():
            for kp in range(2):
                pos = [psum_mm.tile([128, NSL], F32, tag="pqk", name=f"po{i}")
                       for i in range(2)]
                for kk in range(2):
                    k = 2 * kp + kk
                    for g in range(TP):
                        nc.tensor.matmul(
                            pos[kk][:],
                            cf3[0][:, g, k * 128:(k + 1) * 128],
                            wo_sb[:, 2 * g, :],
                            start=(g == 0), stop=False,
                        )
                for kk in range(2):
                    k = 2 * kp + kk
                    for g in range(TP):
                        nc.tensor.matmul(
                            pos[kk][:],
                            cf3[1][:, g, k * 128:(k + 1) * 128],
                            wo_sb[:, 2 * g + 1, :],
                            start=False, stop=(g == TP - 1),
                        )
                for kk in range(2):
                    it = 12 + 2 * kp + kk
                    ot = osb_p.tile([128, NSL], F32, tag="ot")
                    nc.vector.tensor_add(out=ot[:], in0=pos[kk][:], in1=bob[:])
                    nc.sync.dma_start(
                        out=out[it * 128:(it + 1) * 128, :], in_=ot[:])

        # ---- pipeline ----
        # emission order = scheduling priority; E(ic) parts are placed after
        # their gather completes, as fillers for later chunks' attention
        for ic in range(ICH):
            stage_b(ic, xt_cur)
            if ic == 0:
                # wo/bo staged after B(0) q/k: needed first at E(0) in C(3)
                stage_w(wo_sb, wo)
                bo_row = wpool.tile([1, NSL], F32)
                nc.sync.dma_start(out=bo_row[:], in_=bo[None, :])
                bob = wpool.tile([128, NSL], F32)
                nc.gpsimd.partition_broadcast(out_ap=bob[:], in_ap=bo_row[:])
            fillers = {0: [], 1: []}
            if ic + 1 < ICH:
                xt_next, xrbs_next = stage_a_alloc(ic + 1)
                for kt in range(KT):
                    fillers[kt // 4].append(
                        lambda xt=xt_next, xr=xrbs_next, kk=kt:
                            stage_a_part(xt, xr, kk))
            else:
                xt_next = None
            if ic == 3:
                # e(0), e(1) parts fill pair 0; e(2) fills pair 1 (its
                # gather lands mid-C(3)); e3 runs post-loop
                for k in range(4):
                    fillers[0].append(lambda kk=k: stage_e_part(0, kk))
                    fillers[0].append(lambda kk=k: stage_e_part(1, kk))
                    fillers[1].append(lambda kk=k: stage_e_part(2, kk))
            stage_c(ic, fillers)
            xt_cur = xt_next
        stage_e3()


def _build():
    nc = bacc.Bacc("TRN2", target_bir_lowering=False, debug=False,
                   num_devices=N_CORES)
    x = nc.declare_dram_parameter("x", [S, E], F32, isOutput=False).ap()
    wq = nc.declare_dram_parameter("wq", [E, NSL], F32, isOutput=False).ap()
    bq = nc.declare_dram_parameter("bq", [NSL], F32, isOutput=False).ap()
    wk = nc.declare_dram_parameter("wk", [E, NSL], F32, isOutput=False).ap()
    bk = nc.declare_dram_parameter("bk", [NSL], F32, isOutput=False).ap()
    wv = nc.declare_dram_parameter("wv", [E, NSL], F32, isOutput=False).ap()
    bv = nc.declare_dram_parameter("bv", [NSL], F32, isOutput=False).ap()
    wo = nc.declare_dram_parameter("wo", [E, NSL], F32, isOutput=False).ap()
    bo = nc.declare_dram_parameter("bo", [NSL], F32, isOutput=False).ap()
    ident = nc.declare_dram_parameter("ident", [128, 128], BF16,
                                      isOutput=False).ap()
    tri = nc.declare_dram_parameter("tri", [128, 128], BF16,
                                    isOutput=False).ap()
    out = nc.declare_dram_parameter("out", [S, NSL], F32, isOutput=True).ap()

    with tile.TileContext(nc) as tc:
        _emit(nc, tc, (x, wq, bq, wk, bk, wv, bv, wo, bo, ident, tri, out))
    nc.compile()
    return nc


def _consts():
    ident = np.eye(128, dtype=ml_dtypes.bfloat16)
    jl = np.arange(128, dtype=np.int64)[:, None]
    il = np.arange(128, dtype=np.int64)[None, :]
    tri = (il >= jl).astype(ml_dtypes.bfloat16)
    return ident, np.ascontiguousarray(tri)


def kernel(x, Wq, bq, Wk, bk, Wv, bv, Wo, bo, _trace=False, _trace_cores=None):
    if "nc" not in _cache:
        _cache["nc"] = _build()
    nc = _cache["nc"]
    ident, tri = _consts()
    x = np.asarray(x, dtype=np.float32)
    in_maps = []
    for c in range(N_CORES):
        bi, g = divmod(c, TP)
        sl = slice(g * NSL, (g + 1) * NSL)
        in_maps.append({
            "x": np.ascontiguousarray(x[bi]),
            "wq": np.ascontiguousarray(np.asarray(Wq)[:, sl]),
            "bq": np.ascontiguousarray(np.asarray(bq)[sl]),
            "wk": np.ascontiguousarray(np.asarray(Wk)[:, sl]),
            "bk": np.ascontiguousarray(np.asarray(bk)[sl]),
            "wv": np.ascontiguousarray(np.asarray(Wv)[:, sl]),
            "bv": np.ascontiguousarray(np.asarray(bv)[sl]),
            "wo": np.ascontiguousarray(np.asarray(Wo)[:, sl]),
            "bo": np.ascontiguousarray(np.asarray(bo)[sl]),
            "ident": ident,
            "tri": tri,
        })
    res = run_bass_kernel_spmd(
        nc, in_maps, list(range(N_CORES)),
        trace=_trace, trace_cores=_trace_cores)
    out = np.empty((B, S, E), np.float32)
    for c in range(N_CORES):
        bi, g = divmod(c, TP)
        out[bi, :, g * NSL:(g + 1) * NSL] = res.results[c]["out"]
    if _trace:
        _cache["last_result"] = res
    return out



# revision 14
# speedup vs baseline: 1.0556x; 1.0556x over previous
"""Multi-head causal attention (B=2, S=2048, E=1024, H=16, D=64) on 8 trn2 cores.

Sharding (Megatron-style, per hint): data-parallel over batch (2) x
tensor-parallel over heads (4 groups of 4 heads / 256 features).
Core c: batch c//4, head-group c%4.

Per-core device program (SPMD, identical on all cores), pipelined over 4
query chunks of 512, all matmuls in bf16:
  A(ic). DVE-convert x rows to bf16, PE-transpose -> xT for the chunk
  B(ic). qT/kT projections in [n, s] layout; v in natural [s, n] layout
  C(ic). causal attention in transposed-score layout, processed in HEAD
       PAIRS so the two K=64 score matmuls sit in PE row-halves 0:64 /
       64:128 and overlap (PE quadrant packing):
       sT[j,i] = kT_h . qT_h, p = exp(s/8) on ScalarE (bf16),
       causal mask via multiplicative 0/1 tile on diagonal blocks,
       ctxT[d,i] accumulated with v-augmented-with-ones stationary ->
       row 64 of psum = softmax denominator; normalize with
       1/L = exp(-ln L) on ScalarE tables + partition_broadcast + multiply
       (keeps the slow elementwise reciprocal off the DVE queue)
  D(ic). AllGather the normalized ctxT chunk across the 4-core TP group
       (chunks 0-2: one collective; chunk 3: two row-half collectives,
       one per head pair, so the tail collective hides behind pair 1)
  E(ic). out[:, g*256:(g+1)*256] = ctxT_full.T @ Wo[:, slice] + bo[slice]
       (E(ic) is emitted as fillers inside C(ic+1)/C(ic+2) so the
       collectives hide behind compute)
Host only slices inputs and concatenates the 8 disjoint output slices.
"""

import contextlib

import ml_dtypes
import numpy as np

import concourse.mybir as mybir
import concourse.tile as tile
from concourse import bacc
from concourse.bass_utils import run_bass_kernel_spmd

F32 = mybir.dt.float32
BF16 = mybir.dt.bfloat16

B, S, E, H, D = 2, 2048, 1024, 16, 64
N_CORES = 8
TP = 4                 # tensor-parallel degree (head groups per batch)
NSL = E // TP          # 256 features per core
HLOC = H // TP         # 4 heads per core
KT = E // 128          # 8 contraction tiles
IT = S // 128          # 16 sequence tiles
ICH = S // 512         # 4 sequence chunks of 512
SCALE = 1.0 / np.sqrt(D)

REPLICA_GROUPS = [[0, 1, 2, 3], [4, 5, 6, 7]]

_cache: dict = {}


def _emit(nc, tc, prm):
    x, wq, bq, wk, bk, wv, bv, wo, bo, ident, tri, out = prm

    with contextlib.ExitStack() as stack:
        ent = stack.enter_context
        const = ent(tc.tile_pool(name="const", bufs=1))
        wstage = ent(tc.tile_pool(name="wstage", bufs=8))
        wpool = ent(tc.tile_pool(name="wpool", bufs=1))
        xrow_p = ent(tc.tile_pool(name="xrow", bufs=2))
        xt_p = ent(tc.tile_pool(name="xt", bufs=2))
        qkv_p = ent(tc.tile_pool(name="qkv", bufs=1))
        psum_t = ent(tc.tile_pool(name="psum_t", bufs=2, space="PSUM"))
        psum_mm = ent(tc.tile_pool(name="psum_mm", bufs=2, space="PSUM"))
        psum_s = ent(tc.tile_pool(name="psum_s", bufs=2, space="PSUM"))
        psum_c = ent(tc.tile_pool(name="psum_c", bufs=2, space="PSUM"))
        pwork = ent(tc.tile_pool(name="pwork", bufs=6))
        norm_p = ent(tc.tile_pool(name="norm", bufs=2))
        ctxt_p = ent(tc.tile_pool(name="ctxt", bufs=1))
        ctxf_p = ent(tc.tile_pool(name="ctxf", bufs=2))
        osb_p = ent(tc.tile_pool(name="osb", bufs=2))
        dram = ent(tc.tile_pool(name="dram", bufs=1, space="DRAM"))

        # ---- constants ----
        ident_sb = const.tile([128, 128], BF16)
        nc.sync.dma_start(out=ident_sb[:], in_=ident[:])
        tri_sb = const.tile([128, 128], BF16)
        nc.sync.dma_start(out=tri_sb[:], in_=tri[:])
        ones_sb = const.tile([1, D], BF16)
        nc.vector.memset(ones_sb[:], 1.0)

        # ---- persistent activations ----
        qt_sb = qkv_p.tile([128, 2, S], BF16)
        kt_sb = qkv_p.tile([128, 2, S], BF16)
        v_sb = qkv_p.tile([128, IT, HLOC, D + 1], BF16)
        nc.vector.memset(v_sb[:, :, :, D:D + 1], 1.0)
        ctxt_sb = ctxt_p.tile([128, 2, S], BF16)

        # ---- stage A: x rows -> bf16 -> PE transpose ----
        def stage_a_alloc(ic):
            xt_sb = xt_p.tile([128, KT, 512], BF16, tag="xt")
            xrbs = []
            for k4, it in enumerate(range(4 * ic, 4 * ic + 4)):
                xr = xrow_p.tile([128, E], F32, tag="xr", bufs=8)
                xrb = xrow_p.tile([128, E], BF16, tag="xrb", bufs=8)
                for h in range(2):
                    sl = slice(h * (E // 2), (h + 1) * (E // 2))
                    nc.sync.dma_start(
                        out=xr[:, sl], in_=x[it * 128:(it + 1) * 128, sl])
                    nc.vector.tensor_copy(xrb[:, sl], xr[:, sl])
                xrbs.append(xrb)
            return xt_sb, xrbs

        def stage_a_part(xt_sb, xrbs, kt):
            pt = psum_t.tile([128, 512], BF16, tag="pt")
            for k4 in range(4):
                nc.tensor.transpose(
                    pt[:, k4 * 128:(k4 + 1) * 128],
                    xrbs[k4][:, kt * 128:(kt + 1) * 128],
                    ident_sb[:])
            nc.vector.tensor_copy(xt_sb[:, kt, :], pt[:])

        xt_cur, xrbs_cur = stage_a_alloc(0)
        for part in range(KT):
            stage_a_part(xt_cur, xrbs_cur, part)

        # ---- weights: per-kt DMA (gpsimd SWDGE queue) + DVE bf16 convert ----
        def stage_w(w_sb, w_dr):
            w_r = w_dr.rearrange("(t p) n -> p t n", p=128)
            for kt in range(KT):
                wst = wstage.tile([128, NSL], F32, tag="wst", bufs=8)
                nc.gpsimd.dma_start(out=wst[:], in_=w_r[:, kt, :])
                nc.vector.tensor_copy(w_sb[:, kt, :], wst[:])

        wq_sb = wpool.tile([128, KT, NSL], BF16)
        wk_sb = wpool.tile([128, KT, NSL], BF16)
        wv_sb = wpool.tile([128, KT, NSL], BF16)
        wo_sb = wpool.tile([128, KT, NSL], BF16)
        stage_w(wq_sb, wq)
        stage_w(wk_sb, wk)
        stage_w(wv_sb, wv)
        bq_sb = wpool.tile([128, 2], F32)
        bk_sb = wpool.tile([128, 2], F32)
        for b_sb, b_dr in ((bq_sb, bq), (bk_sb, bk)):
            nc.sync.dma_start(out=b_sb[:], in_=b_dr.rearrange("(t p) -> p t", p=128))
        bv_row = wpool.tile([1, NSL], F32)
        nc.sync.dma_start(out=bv_row[:], in_=bv[None, :])
        bvb = wpool.tile([128, NSL], F32)
        nc.gpsimd.partition_broadcast(out_ap=bvb[:], in_ap=bv_row[:])

        # ---- stage B: projections ----
        def stage_b(ic, xt_sb):
            for (w_sb, b_sb, o_sb) in ((wq_sb, bq_sb, qt_sb),
                                       (wk_sb, bk_sb, kt_sb)):
                osl = slice(ic * 512, (ic + 1) * 512)
                for nt in range(2):
                    pm = psum_mm.tile([128, 512], F32, tag="pqk")
                    for kt in range(KT):
                        nc.tensor.matmul(
                            pm[:],
                            w_sb[:, kt, nt * 128:(nt + 1) * 128],
                            xt_sb[:, kt, :],
                            start=(kt == 0), stop=(kt == KT - 1),
                        )
                    nc.vector.tensor_scalar_add(
                        out=o_sb[:, nt, osl],
                        in0=pm[:], scalar1=b_sb[:, nt:nt + 1])
            for k4, it in enumerate(range(4 * ic, 4 * ic + 4)):
                pv = psum_mm.tile([128, NSL], F32, tag="pqk")
                for kt in range(KT):
                    nc.tensor.matmul(
                        pv[:],
                        xt_sb[:, kt, k4 * 128:(k4 + 1) * 128],
                        wv_sb[:, kt, :],
                        start=(kt == 0), stop=(kt == KT - 1),
                    )
                nc.vector.tensor_add(
                    out=v_sb[:, it, :, 0:D],
                    in0=pv[:].rearrange("p (h d) -> p h d", d=D),
                    in1=bvb[:].rearrange("p (h d) -> p h d", d=D))

        # DRAM bounce buffers for the chunked allgather. Chunks 0+1 share one
        # buffer and one merged collective (per-op fixed cost ~8-10us, so
        # fewer, bigger gathers win); chunk 2 is alone; chunk 3 goes as two
        # row-half gathers so the tail collective is small.
        cc_in01 = dram.tile([NSL, 2 * 512], BF16)
        cc_out01 = dram.tile([TP, NSL, 2 * 512], BF16)
        cc_in2 = dram.tile([NSL, 512], BF16)
        cc_out2 = dram.tile([TP, NSL, 512], BF16)
        cc_in3 = dram.tile([NSL, 512], BF16)
        cc3_out = [dram.tile([TP, 128, 512], BF16, name=f"cc3_out{h}")
                   for h in range(2)]

        def cc_in_ap(ic, nt):
            rsl = slice(nt * 128, (nt + 1) * 128)
            if ic <= 1:
                return cc_in01[rsl, ic * 512:(ic + 1) * 512]
            if ic == 2:
                return cc_in2[rsl, :]
            return cc_in3[rsl, :]

        def stage_d01():
            nc.gpsimd.collective_compute(
                "AllGather", mybir.AluOpType.bypass,
                replica_groups=REPLICA_GROUPS,
                ins=[cc_in01.opt()], outs=[cc_out01.opt()],
            )

        def stage_d2():
            nc.gpsimd.collective_compute(
                "AllGather", mybir.AluOpType.bypass,
                replica_groups=REPLICA_GROUPS,
                ins=[cc_in2.opt()], outs=[cc_out2.opt()],
            )

        def stage_d3_half(nt):
            nc.gpsimd.collective_compute(
                "AllGather", mybir.AluOpType.bypass,
                replica_groups=REPLICA_GROUPS,
                ins=[cc_in3[nt * 128:(nt + 1) * 128, :].opt()],
                outs=[cc3_out[nt].opt()],
            )

        # ---- stage C: causal attention, head pairs ----
        def stage_c(ic, fillers):
            i0 = ic * 512
            njt = 4 * (ic + 1)

            def emit_s(h, jt):
                # diagonal j-tiles only need columns i_local >= 128*dt_
                nt = h // 2
                base = (h % 2) * D
                dt_ = jt - 4 * ic
                c0 = max(dt_, 0) * 128
                ps = psum_s.tile([128, 512], F32, tag="ps")
                nc.tensor.matmul(
                    ps[:, c0:],
                    kt_sb[base:base + D, nt, jt * 128:(jt + 1) * 128],
                    qt_sb[base:base + D, nt, i0 + c0:i0 + 512],
                    start=True, stop=True,
                )
                pw = pwork.tile([128, 512], BF16, tag="pw")
                nc.scalar.activation(
                    out=pw[:, c0:], in_=ps[:, c0:],
                    func=mybir.ActivationFunctionType.Exp, scale=float(SCALE))
                if dt_ >= 0:
                    nc.vector.tensor_mul(
                        pw[:, c0:c0 + 128], pw[:, c0:c0 + 128], tri_sb[:])
                return pw

            def emit_ctx(h, jt, pc, pw):
                c0 = max(jt - 4 * ic, 0) * 128
                nc.tensor.matmul(
                    pc[:, c0:],
                    v_sb[:, jt, h, :],
                    pw[:, c0:],
                    start=(jt == 0), stop=(jt == njt - 1),
                )

            for hp in range(2):
                h0, h1 = 2 * hp, 2 * hp + 1
                nt = hp
                pcs = [psum_c.tile([D + 1, 512], F32, tag="pc", name=f"pc{i}")
                       for i in range(2)]
                # score matmuls for the pair are adjacent: K=64 stationaries
                # at PE row-halves 0:64 / 64:128 execute concurrently
                pws = [emit_s(h0, 0), emit_s(h1, 0)]
                for jt in range(1, njt):
                    nxt = [emit_s(h0, jt), emit_s(h1, jt)]
                    emit_ctx(h0, jt - 1, pcs[0], pws[0])
                    emit_ctx(h1, jt - 1, pcs[1], pws[1])
                    pws = nxt
                emit_ctx(h0, njt - 1, pcs[0], pws[0])
                emit_ctx(h1, njt - 1, pcs[1], pws[1])
                for idx, h in enumerate((h0, h1)):
                    base = (h % 2) * D
                    # 1/L on DVE (fast approx; L-row first copied to a
                    # partition-0 SBUF tile — the custom-DVE op mishandles
                    # partition-offset inputs), then broadcast down 64
                    # partitions with a 1-row-stationary PE matmul
                    lrow = norm_p.tile([1, 512], F32, tag="lrow")
                    nc.vector.tensor_copy(lrow[:], pcs[idx][D:D + 1, :])
                    lrec = norm_p.tile([1, 512], F32, tag="lrec")
                    nc.vector.reciprocal_approx_fast(
                        out=lrec[:], in_=lrow[:])
                    lrecb = norm_p.tile([1, 512], BF16, tag="lrecb")
                    nc.vector.tensor_copy(lrecb[:], lrec[:])
                    lb = psum_s.tile([128, 512], F32, tag="ps", name="lb")
                    nc.tensor.matmul(
                        lb[0:D, :], ones_sb[:], lrecb[:],
                        start=True, stop=True)
                    lbs = norm_p.tile([D, 512], F32, tag="lbs")
                    nc.vector.tensor_copy(lbs[:], lb[0:D, :])
                    nc.vector.tensor_mul(
                        ctxt_sb[base:base + D, nt, i0:i0 + 512],
                        pcs[idx][0:D, :], lbs[:])
                nc.sync.dma_start(
                    out=cc_in_ap(ic, nt),
                    in_=ctxt_sb[:, nt, i0:i0 + 512])
                if ic == ICH - 1:
                    stage_d3_half(nt)
                    stage_e3_load(nt)
                elif nt == 1:
                    if ic == 1:
                        stage_d01()
                        stage_e01_load()
                    elif ic == 2:
                        stage_d2()
                        stage_e2_load()
                for f in fillers.get(hp, []):
                    f()

        # ---- stage E: out-projection ----
        # e-loads go on the gpsimd queue: they block waiting for their
        # gather, and gpsimd is the one queue where that's harmless (the CC
        # stream is serial, so the next collective couldn't start earlier
        # anyway). Keeping them off scalar/sync keeps exp and x-loads flowing.
        efs = {}

        def stage_e01_load():
            ctxf_sb = ctxf_p.tile([128, KT, 2 * 512], BF16, tag="ctxf01",
                                  bufs=1)
            nc.gpsimd.dma_start(
                out=ctxf_sb[:],
                in_=cc_out01.rearrange("g (t p) i -> p (g t) i", p=128))
            efs["01"] = ctxf_sb

        def stage_e2_load():
            ctxf_sb = ctxf_p.tile([128, KT, 512], BF16, tag="ctxf2", bufs=1)
            nc.gpsimd.dma_start(
                out=ctxf_sb[:],
                in_=cc_out2.rearrange("g (t p) i -> p (g t) i", p=128))
            efs["2"] = ctxf_sb

        def stage_e_part(ic, k):
            ctxf_sb = efs["01"] if ic <= 1 else efs["2"]
            off = ic * 512 if ic <= 1 else 0
            it = 4 * ic + k
            po = psum_mm.tile([128, NSL], F32, tag="pqk")
            for ct in range(KT):
                nc.tensor.matmul(
                    po[:],
                    ctxf_sb[:, ct, off + k * 128:off + (k + 1) * 128],
                    wo_sb[:, ct, :],
                    start=(ct == 0), stop=(ct == KT - 1),
                )
            ot = osb_p.tile([128, NSL], F32, tag="ot")
            nc.vector.tensor_add(out=ot[:], in0=po[:], in1=bob[:])
            nc.sync.dma_start(
                out=out[it * 128:(it + 1) * 128, :], in_=ot[:])

        # last chunk: two row-half gathers, accumulate evens then odds so the
        # second collective hides behind the even-half matmuls
        cf3 = [None, None]

        def stage_e3_load(nt):
            cf = ctxf_p.tile([128, TP, 512], BF16, tag=f"cf3_{nt}", bufs=1)
            nc.gpsimd.dma_start(
                out=cf[:], in_=cc3_out[nt].rearrange("g p i -> p g i"))
            cf3[nt] = cf

        def stage_e3():
            for kp in range(2):
                pos = [psum_mm.tile([128, NSL], F32, tag="pqk", name=f"po{i}")
                       for i in range(2)]
                for kk in range(2):
                    k = 2 * kp + kk
                    for g in range(TP):
                        nc.tensor.matmul(
                            pos[kk][:],
                            cf3[0][:, g, k * 128:(k + 1) * 128],
                            wo_sb[:, 2 * g, :],
                            start=(g == 0), stop=False,
                        )
                for kk in range(2):
                    k = 2 * kp + kk
                    for g in range(TP):
                        nc.tensor.matmul(
                            pos[kk][:],
                            cf3[1][:, g, k * 128:(k + 1) * 128],
                            wo_sb[:, 2 * g + 1, :],
                            start=False, stop=(g == TP - 1),
                        )
                for kk in range(2):
                    it = 12 + 2 * kp + kk
                    ot = osb_p.tile([128, NSL], F32, tag="ot")
                    nc.vector.tensor_add(out=ot[:], in0=pos[kk][:], in1=bob[:])
                    nc.sync.dma_start(
                        out=out[it * 128:(it + 1) * 128, :], in_=ot[:])

        # ---- pipeline ----
        # emission order = scheduling priority; E(ic) parts are placed after
        # their gather completes, as fillers for later chunks' attention
        for ic in range(ICH):
            stage_b(ic, xt_cur)
            if ic == 0:
                # wo/bo staged after B(0) q/k: needed first at E(0) in C(3)
                stage_w(wo_sb, wo)
                bo_row = wpool.tile([1, NSL], F32)
                nc.sync.dma_start(out=bo_row[:], in_=bo[None, :])
                bob = wpool.tile([128, NSL], F32)
                nc.gpsimd.partition_broadcast(out_ap=bob[:], in_ap=bo_row[:])
            fillers = {0: [], 1: []}
            if ic + 1 < ICH:
                xt_next, xrbs_next = stage_a_alloc(ic + 1)
                for kt in range(KT):
                    fillers[kt // 4].append(
                        lambda xt=xt_next, xr=xrbs_next, kk=kt:
                            stage_a_part(xt, xr, kk))
            else:
                xt_next = None
            if ic == 3:
                # e(0), e(1) parts fill pair 0; e(2) fills pair 1 (its
                # gather lands mid-C(3)); e3 runs post-loop
                for k in range(4):
                    fillers[0].append(lambda kk=k: stage_e_part(0, kk))
                    fillers[0].append(lambda kk=k: stage_e_part(1, kk))
                    fillers[1].append(lambda kk=k: stage_e_part(2, kk))
            stage_c(ic, fillers)
            xt_cur = xt_next
        stage_e3()


def _build():
    nc = bacc.Bacc("TRN2", target_bir_lowering=False, debug=False,
                   num_devices=N_CORES)
    x = nc.declare_dram_parameter("x", [S, E], F32, isOutput=False).ap()
    wq = nc.declare_dram_parameter("wq", [E, NSL], F32, isOutput=False).ap()
    bq = nc.declare_dram_parameter("bq", [NSL], F32, isOutput=False).ap()
    wk = nc.declare_dram_parameter("wk", [E, NSL], F32, isOutput=False).ap()
    bk = nc.declare_dram_parameter("bk", [NSL], F32, isOutput=False).ap()
    wv = nc.declare_dram_parameter("wv", [E, NSL], F32, isOutput=False).ap()
    bv = nc.declare_dram_parameter("bv", [NSL], F32, isOutput=False).ap()
    wo = nc.declare_dram_parameter("wo", [E, NSL], F32, isOutput=False).ap()
    bo = nc.declare_dram_parameter("bo", [NSL], F32, isOutput=False).ap()
    ident = nc.declare_dram_parameter("ident", [128, 128], BF16,
                                      isOutput=False).ap()
    tri = nc.declare_dram_parameter("tri", [128, 128], BF16,
                                    isOutput=False).ap()
    out = nc.declare_dram_parameter("out", [S, NSL], F32, isOutput=True).ap()

    with tile.TileContext(nc) as tc:
        _emit(nc, tc, (x, wq, bq, wk, bk, wv, bv, wo, bo, ident, tri, out))
    nc.compile()
    return nc


def _consts():
    ident = np.eye(128, dtype=ml_dtypes.bfloat16)
    jl = np.arange(128, dtype=np.int64)[:, None]
    il = np.arange(128, dtype=np.int64)[None, :]
    tri = (il >= jl).astype(ml_dtypes.bfloat16)
    return ident, np.ascontiguousarray(tri)


def kernel(x, Wq, bq, Wk, bk, Wv, bv, Wo, bo, _trace=False, _trace_cores=None):
    if "nc" not in _cache:
        _cache["nc"] = _build()
    nc = _cache["nc"]
    ident, tri = _consts()
    x = np.asarray(x, dtype=np.float32)
    in_maps = []
    for c in range(N_CORES):
        bi, g = divmod(c, TP)
        sl = slice(g * NSL, (g + 1) * NSL)
        in_maps.append({
            "x": np.ascontiguousarray(x[bi]),
            "wq": np.ascontiguousarray(np.asarray(Wq)[:, sl]),
            "bq": np.ascontiguousarray(np.asarray(bq)[sl]),
            "wk": np.ascontiguousarray(np.asarray(Wk)[:, sl]),
            "bk": np.ascontiguousarray(np.asarray(bk)[sl]),
            "wv": np.ascontiguousarray(np.asarray(Wv)[:, sl]),
            "bv": np.ascontiguousarray(np.asarray(bv)[sl]),
            "wo": np.ascontiguousarray(np.asarray(Wo)[:, sl]),
            "bo": np.ascontiguousarray(np.asarray(bo)[sl]),
            "ident": ident,
            "tri": tri,
        })
    res = run_bass_kernel_spmd(
        nc, in_maps, list(range(N_CORES)),
        trace=_trace, trace_cores=_trace_cores)
    out = np.empty((B, S, E), np.float32)
    for c in range(N_CORES):
        bi, g = divmod(c, TP)
        out[bi, :, g * NSL:(g + 1) * NSL] = res.results[c]["out"]
    if _trace:
        _cache["last_result"] = res
    return out



# revision 25
# speedup vs baseline: 1.1584x; 1.0974x over previous
"""Multi-head causal attention (B=2, S=2048, E=1024, H=16, D=64) on 8 trn2 cores.

Sharding (Megatron-style, per hint): data-parallel over batch (2) x
tensor-parallel over heads (4 groups of 4 heads / 256 features).
Core c: batch c//4, head-group c%4.

Per-core device program (SPMD, identical on all cores), pipelined over 4
query chunks of 512, all matmuls in bf16:
  A(ic). DVE-convert x rows to bf16, PE-transpose -> xT for the chunk
  B(ic). qT/kT projections in [n, s] layout; v in natural [s, n] layout
  C(ic). causal attention in transposed-score layout, processed in HEAD
       PAIRS so the two K=64 score matmuls sit in PE row-halves 0:64 /
       64:128 and overlap (PE quadrant packing):
       sT[j,i] = kT_h . qT_h, p = exp(s/8) on ScalarE (bf16),
       causal mask via multiplicative 0/1 tile on diagonal blocks,
       ctxT[d,i] accumulated with v-augmented-with-ones stationary ->
       row 64 of psum = softmax denominator; normalize with
       1/L = exp(-ln L) on ScalarE tables + partition_broadcast + multiply
       (keeps the slow elementwise reciprocal off the DVE queue)
  D(ic). AllGather the normalized ctxT chunk across the 4-core TP group
       (chunks 0-2: one collective; chunk 3: two row-half collectives,
       one per head pair, so the tail collective hides behind pair 1)
  E(ic). out[:, g*256:(g+1)*256] = ctxT_full.T @ Wo[:, slice] + bo[slice]
       (E(ic) is emitted as fillers inside C(ic+1)/C(ic+2) so the
       collectives hide behind compute)
Host only slices inputs and concatenates the 8 disjoint output slices.
"""

import contextlib

import ml_dtypes
import numpy as np

import concourse.mybir as mybir
import concourse.tile as tile
from concourse import bacc
from concourse.bass_utils import run_bass_kernel_spmd

F32 = mybir.dt.float32
BF16 = mybir.dt.bfloat16

B, S, E, H, D = 2, 2048, 1024, 16, 64
N_CORES = 8
TP = 4                 # tensor-parallel degree (head groups per batch)
NSL = E // TP          # 256 features per core
HLOC = H // TP         # 4 heads per core
KT = E // 128          # 8 contraction tiles
IT = S // 128          # 16 sequence tiles
ICH = S // 512         # 4 sequence chunks of 512
SCALE = 1.0 / np.sqrt(D)

REPLICA_GROUPS = [[0, 1, 2, 3], [4, 5, 6, 7]]

_cache: dict = {}


def _emit(nc, tc, prm):
    x, wq, bq, wk, bk, wv, bv, wo, bo, ident, tri, out = prm

    with contextlib.ExitStack() as stack:
        ent = stack.enter_context
        const = ent(tc.tile_pool(name="const", bufs=1))
        wpool = ent(tc.tile_pool(name="wpool", bufs=1))
        xrow_p = ent(tc.tile_pool(name="xrow", bufs=2))
        xt_p = ent(tc.tile_pool(name="xt", bufs=2))
        qkv_p = ent(tc.tile_pool(name="qkv", bufs=1))
        psum_t = ent(tc.tile_pool(name="psum_t", bufs=2, space="PSUM"))
        psum_mm = ent(tc.tile_pool(name="psum_mm", bufs=2, space="PSUM"))
        psum_s = ent(tc.tile_pool(name="psum_s", bufs=2, space="PSUM"))
        psum_c = ent(tc.tile_pool(name="psum_c", bufs=2, space="PSUM"))
        pwork = ent(tc.tile_pool(name="pwork", bufs=6))
        norm_p = ent(tc.tile_pool(name="norm", bufs=2))
        ctxt_p = ent(tc.tile_pool(name="ctxt", bufs=1))
        ctxf_p = ent(tc.tile_pool(name="ctxf", bufs=2))
        osb_p = ent(tc.tile_pool(name="osb", bufs=2))
        dram = ent(tc.tile_pool(name="dram", bufs=1, space="DRAM"))

        # ---- constants ----
        ident_sb = const.tile([128, 128], BF16)
        nc.sync.dma_start(out=ident_sb[:], in_=ident[:])
        trineg_sb = const.tile([128, 128], BF16)
        nc.sync.dma_start(out=trineg_sb[:], in_=tri[:])
        ones_sb = const.tile([1, D], BF16)
        nc.vector.memset(ones_sb[:], 1.0)

        # ---- weights (already bf16 on host): straight HWDGE loads ----
        wq_sb = wpool.tile([128, KT, NSL], BF16)
        wk_sb = wpool.tile([128, KT, NSL], BF16)
        wv_sb = wpool.tile([128, KT, NSL], BF16)
        wo_sb = wpool.tile([128, KT, NSL], BF16)

        def stage_w(w_sb, w_dr):
            w_r = w_dr.rearrange("(t p) n -> p t n", p=128)
            for kt in range(KT):
                nc.scalar.dma_start(out=w_sb[:, kt, :], in_=w_r[:, kt, :])

        stage_w(wq_sb, wq)
        stage_w(wk_sb, wk)
        stage_w(wv_sb, wv)
        stage_w(wo_sb, wo)
        bo_row = wpool.tile([1, NSL], F32)
        nc.sync.dma_start(out=bo_row[:], in_=bo[None, :])
        bob = wpool.tile([128, NSL], F32)
        nc.gpsimd.partition_broadcast(out_ap=bob[:], in_ap=bo_row[:])

        # ---- persistent activations ----
        qt_sb = qkv_p.tile([128, 2, S], BF16)
        kt_sb = qkv_p.tile([128, 2, S], BF16)
        v_sb = qkv_p.tile([128, IT, HLOC, D + 1], BF16)
        nc.vector.memset(v_sb[:, :, :, D:D + 1], 1.0)
        ctxt_sb = ctxt_p.tile([128, 2, S], BF16)

        # ---- stage A: x rows (bf16 on host) -> PE transpose ----
        def stage_a_alloc(ic):
            xt_sb = xt_p.tile([128, KT, 512], BF16, tag="xt")
            xrbs = []
            for k4, it in enumerate(range(4 * ic, 4 * ic + 4)):
                xrb = xrow_p.tile([128, E], BF16, tag="xrb", bufs=8)
                for h in range(2):
                    sl = slice(h * (E // 2), (h + 1) * (E // 2))
                    nc.sync.dma_start(
                        out=xrb[:, sl], in_=x[it * 128:(it + 1) * 128, sl])
                xrbs.append(xrb)
            return xt_sb, xrbs

        def stage_a_part(xt_sb, xrbs, kt):
            pt = psum_t.tile([128, 512], BF16, tag="pt")
            for k4 in range(4):
                nc.tensor.transpose(
                    pt[:, k4 * 128:(k4 + 1) * 128],
                    xrbs[k4][:, kt * 128:(kt + 1) * 128],
                    ident_sb[:])
            nc.vector.tensor_copy(xt_sb[:, kt, :], pt[:])

        xt_cur, xrbs_cur = stage_a_alloc(0)
        for part in range(KT):
            stage_a_part(xt_cur, xrbs_cur, part)

        bq_sb = wpool.tile([128, 2], F32)
        bk_sb = wpool.tile([128, 2], F32)
        for b_sb, b_dr in ((bq_sb, bq), (bk_sb, bk)):
            nc.sync.dma_start(out=b_sb[:], in_=b_dr.rearrange("(t p) -> p t", p=128))
        bv_row = wpool.tile([1, NSL], F32)
        nc.sync.dma_start(out=bv_row[:], in_=bv[None, :])
        bvb = wpool.tile([128, NSL], F32)
        nc.gpsimd.partition_broadcast(out_ap=bvb[:], in_ap=bv_row[:])

        # ---- stage B: projections ----
        def stage_b(ic, xt_sb):
            for (w_sb, b_sb, o_sb) in ((wq_sb, bq_sb, qt_sb),
                                       (wk_sb, bk_sb, kt_sb)):
                osl = slice(ic * 512, (ic + 1) * 512)
                for nt in range(2):
                    pm = psum_mm.tile([128, 512], F32, tag="pqk")
                    for kt in range(KT):
                        nc.tensor.matmul(
                            pm[:],
                            w_sb[:, kt, nt * 128:(nt + 1) * 128],
                            xt_sb[:, kt, :],
                            start=(kt == 0), stop=(kt == KT - 1),
                        )
                    nc.vector.tensor_scalar_add(
                        out=o_sb[:, nt, osl],
                        in0=pm[:], scalar1=b_sb[:, nt:nt + 1])
            for k4, it in enumerate(range(4 * ic, 4 * ic + 4)):
                pv = psum_mm.tile([128, NSL], F32, tag="pqk")
                for kt in range(KT):
                    nc.tensor.matmul(
                        pv[:],
                        xt_sb[:, kt, k4 * 128:(k4 + 1) * 128],
                        wv_sb[:, kt, :],
                        start=(kt == 0), stop=(kt == KT - 1),
                    )
                nc.vector.tensor_add(
                    out=v_sb[:, it, :, 0:D],
                    in0=pv[:].rearrange("p (h d) -> p h d", d=D),
                    in1=bvb[:].rearrange("p (h d) -> p h d", d=D))

        # DRAM bounce buffers for the chunked allgather. Chunks 0+1 share one
        # buffer and one merged collective (per-op fixed cost ~8-10us, so
        # fewer, bigger gathers win); chunk 2 is alone; chunk 3 goes as two
        # row-half gathers so the tail collective is small.
        cc_in01 = dram.tile([NSL, 2 * 512], BF16)
        cc_out01 = dram.tile([TP, NSL, 2 * 512], BF16)
        cc_in2 = dram.tile([NSL, 512], BF16)
        cc_out2 = dram.tile([TP, NSL, 512], BF16)
        cc_in3 = dram.tile([NSL, 512], BF16)
        cc3_out = [dram.tile([TP, 128, 512], BF16, name=f"cc3_out{h}")
                   for h in range(2)]

        def cc_in_ap(ic, nt):
            rsl = slice(nt * 128, (nt + 1) * 128)
            if ic <= 1:
                return cc_in01[rsl, ic * 512:(ic + 1) * 512]
            if ic == 2:
                return cc_in2[rsl, :]
            return cc_in3[rsl, :]

        def stage_d01():
            nc.gpsimd.collective_compute(
                "AllGather", mybir.AluOpType.bypass,
                replica_groups=REPLICA_GROUPS,
                ins=[cc_in01.opt()], outs=[cc_out01.opt()],
            )

        def stage_d2():
            nc.gpsimd.collective_compute(
                "AllGather", mybir.AluOpType.bypass,
                replica_groups=REPLICA_GROUPS,
                ins=[cc_in2.opt()], outs=[cc_out2.opt()],
            )

        def stage_d3_half(nt):
            nc.gpsimd.collective_compute(
                "AllGather", mybir.AluOpType.bypass,
                replica_groups=REPLICA_GROUPS,
                ins=[cc_in3[nt * 128:(nt + 1) * 128, :].opt()],
                outs=[cc3_out[nt].opt()],
            )

        # ---- stage C: causal attention, head pairs ----
        def stage_c(ic, fillers):
            i0 = ic * 512
            njt = 4 * (ic + 1)

            def emit_s(h, jt):
                # diagonal j-tiles only need columns i_local >= 128*dt_
                nt = h // 2
                base = (h % 2) * D
                dt_ = jt - 4 * ic
                diag = dt_ >= 0
                c0 = max(dt_, 0) * 128
                ps = psum_s.tile([128, 512], F32, tag="ps")
                nc.tensor.matmul(
                    ps[:, c0:],
                    kt_sb[base:base + D, nt, jt * 128:(jt + 1) * 128],
                    qt_sb[base:base + D, nt, i0 + c0:i0 + 512],
                    start=True, stop=not diag,
                )
                if diag:
                    # causal mask folded into psum: += -1e9 upper triangle
                    # (53ns PE matmul instead of a ~700ns DVE multiply)
                    nc.tensor.matmul(
                        ps[:, c0:c0 + 128], trineg_sb[:], ident_sb[:],
                        start=False, stop=True,
                    )
                pw = pwork.tile([128, 512], BF16, tag="pw")
                nc.scalar.activation(
                    out=pw[:, c0:], in_=ps[:, c0:],
                    func=mybir.ActivationFunctionType.Exp, scale=float(SCALE))
                return pw

            def emit_ctx(h, jt, pc, pw):
                c0 = max(jt - 4 * ic, 0) * 128
                nc.tensor.matmul(
                    pc[:, c0:],
                    v_sb[:, jt, h, :],
                    pw[:, c0:],
                    start=(jt == 0), stop=(jt == njt - 1),
                )

            for hp in range(2):
                h0, h1 = 2 * hp, 2 * hp + 1
                nt = hp
                pcs = [psum_c.tile([D + 1, 512], F32, tag="pc", name=f"pc{i}")
                       for i in range(2)]
                # score matmuls for the pair are adjacent: K=64 stationaries
                # at PE row-halves 0:64 / 64:128 execute concurrently
                pws = [emit_s(h0, 0), emit_s(h1, 0)]
                for jt in range(1, njt):
                    nxt = [emit_s(h0, jt), emit_s(h1, jt)]
                    emit_ctx(h0, jt - 1, pcs[0], pws[0])
                    emit_ctx(h1, jt - 1, pcs[1], pws[1])
                    pws = nxt
                emit_ctx(h0, njt - 1, pcs[0], pws[0])
                emit_ctx(h1, njt - 1, pcs[1], pws[1])
                for idx, h in enumerate((h0, h1)):
                    base = (h % 2) * D
                    # 1/L on DVE (fast approx; L-row first copied to a
                    # partition-0 SBUF tile — the custom-DVE op mishandles
                    # partition-offset inputs), then broadcast down 64
                    # partitions with a 1-row-stationary PE matmul
                    lrow = norm_p.tile([1, 512], F32, tag="lrow")
                    nc.vector.tensor_copy(lrow[:], pcs[idx][D:D + 1, :])
                    lrec = norm_p.tile([1, 512], F32, tag="lrec")
                    nc.vector.reciprocal_approx_fast(
                        out=lrec[:], in_=lrow[:])
                    lrecb = norm_p.tile([1, 512], BF16, tag="lrecb")
                    nc.vector.tensor_copy(lrecb[:], lrec[:])
                    lb = psum_s.tile([128, 512], F32, tag="ps", name="lb")
                    nc.tensor.matmul(
                        lb[0:D, :], ones_sb[:], lrecb[:],
                        start=True, stop=True)
                    lbs = norm_p.tile([D, 512], F32, tag="lbs")
                    nc.vector.tensor_copy(lbs[:], lb[0:D, :])
                    nc.vector.tensor_mul(
                        ctxt_sb[base:base + D, nt, i0:i0 + 512],
                        pcs[idx][0:D, :], lbs[:])
                nc.sync.dma_start(
                    out=cc_in_ap(ic, nt),
                    in_=ctxt_sb[:, nt, i0:i0 + 512])
                if ic == ICH - 1:
                    stage_d3_half(nt)
                    stage_e3_load(nt)
                elif nt == 1:
                    if ic == 1:
                        stage_d01()
                        stage_e01_load()
                    elif ic == 2:
                        stage_d2()
                        stage_e2_load()
                for f in fillers.get(hp, []):
                    f()

        # ---- stage E: out-projection ----
        # e-loads go on the gpsimd queue: they block waiting for their
        # gather, and gpsimd is the one queue where that's harmless (the CC
        # stream is serial, so the next collective couldn't start earlier
        # anyway). Keeping them off scalar/sync keeps exp and x-loads flowing.
        efs = {}

        def stage_e01_load():
            ctxf_sb = ctxf_p.tile([128, KT, 2 * 512], BF16, tag="ctxf01",
                                  bufs=1)
            nc.gpsimd.dma_start(
                out=ctxf_sb[:],
                in_=cc_out01.rearrange("g (t p) i -> p (g t) i", p=128))
            efs["01"] = ctxf_sb

        def stage_e2_load():
            ctxf_sb = ctxf_p.tile([128, KT, 512], BF16, tag="ctxf2", bufs=1)
            nc.gpsimd.dma_start(
                out=ctxf_sb[:],
                in_=cc_out2.rearrange("g (t p) i -> p (g t) i", p=128))
            efs["2"] = ctxf_sb

        def stage_e_part(ic, k):
            ctxf_sb = efs["01"] if ic <= 1 else efs["2"]
            off = ic * 512 if ic <= 1 else 0
            it = 4 * ic + k
            po = psum_mm.tile([128, NSL], F32, tag="pqk")
            for ct in range(KT):
                nc.tensor.matmul(
                    po[:],
                    ctxf_sb[:, ct, off + k * 128:off + (k + 1) * 128],
                    wo_sb[:, ct, :],
                    start=(ct == 0), stop=(ct == KT - 1),
                )
            ot = osb_p.tile([128, NSL], F32, tag="ot")
            nc.vector.tensor_add(out=ot[:], in0=po[:], in1=bob[:])
            nc.sync.dma_start(
                out=out[it * 128:(it + 1) * 128, :], in_=ot[:])

        # last chunk: two row-half gathers, accumulate evens then odds so the
        # second collective hides behind the even-half matmuls
        cf3 = [None, None]

        def stage_e3_load(nt):
            cf = ctxf_p.tile([128, TP, 512], BF16, tag=f"cf3_{nt}", bufs=1)
            nc.gpsimd.dma_start(
                out=cf[:], in_=cc3_out[nt].rearrange("g p i -> p g i"))
            cf3[nt] = cf

        def stage_e3():
            for kp in range(2):
                pos = [psum_mm.tile([128, NSL], F32, tag="pqk", name=f"po{i}")
                       for i in range(2)]
                for kk in range(2):
                    k = 2 * kp + kk
                    for g in range(TP):
                        nc.tensor.matmul(
                            pos[kk][:],
                            cf3[0][:, g, k * 128:(k + 1) * 128],
                            wo_sb[:, 2 * g, :],
                            start=(g == 0), stop=False,
                        )
                for kk in range(2):
                    k = 2 * kp + kk
                    for g in range(TP):
                        nc.tensor.matmul(
                            pos[kk][:],
                            cf3[1][:, g, k * 128:(k + 1) * 128],
                            wo_sb[:, 2 * g + 1, :],
                            start=False, stop=(g == TP - 1),
                        )
                for kk in range(2):
                    it = 12 + 2 * kp + kk
                    ot = osb_p.tile([128, NSL], F32, tag="ot")
                    nc.vector.tensor_add(out=ot[:], in0=pos[kk][:], in1=bob[:])
                    nc.sync.dma_start(
                        out=out[it * 128:(it + 1) * 128, :], in_=ot[:])

        # ---- pipeline ----
        # emission order = scheduling priority; E(ic) parts are placed after
        # their gather completes, as fillers for later chunks' attention
        for ic in range(ICH):
            stage_b(ic, xt_cur)
            fillers = {0: [], 1: []}
            if ic + 1 < ICH:
                xt_next, xrbs_next = stage_a_alloc(ic + 1)
                for kt in range(KT):
                    fillers[kt // 4].append(
                        lambda xt=xt_next, xr=xrbs_next, kk=kt:
                            stage_a_part(xt, xr, kk))
            else:
                xt_next = None
            if ic == 3:
                # all deferred e-parts fill pair 0, so pair 1 (whose gather
                # is the exposed tail collective) finishes as soon as
                # possible after pair 0
                for k in range(4):
                    fillers[0].append(lambda kk=k: stage_e_part(0, kk))
                    fillers[0].append(lambda kk=k: stage_e_part(1, kk))
                    fillers[0].append(lambda kk=k: stage_e_part(2, kk))
            stage_c(ic, fillers)
            xt_cur = xt_next
        stage_e3()


def _build():
    nc = bacc.Bacc("TRN2", target_bir_lowering=False, debug=False,
                   num_devices=N_CORES)
    x = nc.declare_dram_parameter("x", [S, E], BF16, isOutput=False).ap()
    wq = nc.declare_dram_parameter("wq", [E, NSL], BF16, isOutput=False).ap()
    bq = nc.declare_dram_parameter("bq", [NSL], F32, isOutput=False).ap()
    wk = nc.declare_dram_parameter("wk", [E, NSL], BF16, isOutput=False).ap()
    bk = nc.declare_dram_parameter("bk", [NSL], F32, isOutput=False).ap()
    wv = nc.declare_dram_parameter("wv", [E, NSL], BF16, isOutput=False).ap()
    bv = nc.declare_dram_parameter("bv", [NSL], F32, isOutput=False).ap()
    wo = nc.declare_dram_parameter("wo", [E, NSL], BF16, isOutput=False).ap()
    bo = nc.declare_dram_parameter("bo", [NSL], F32, isOutput=False).ap()
    ident = nc.declare_dram_parameter("ident", [128, 128], BF16,
                                      isOutput=False).ap()
    tri = nc.declare_dram_parameter("tri", [128, 128], BF16,
                                    isOutput=False).ap()
    out = nc.declare_dram_parameter("out", [S, NSL], F32, isOutput=True).ap()

    with tile.TileContext(nc) as tc:
        _emit(nc, tc, (x, wq, bq, wk, bk, wv, bv, wo, bo, ident, tri, out))
    nc.compile()
    return nc


def _consts():
    ident = np.eye(128, dtype=ml_dtypes.bfloat16)
    a = np.arange(128, dtype=np.int64)[:, None]
    b = np.arange(128, dtype=np.int64)[None, :]
    # additive causal mask, oriented for ps[j,i] += trineg[i,j] via
    # matmul(ps, trineg, ident): keep (0) where a>=b, else -1e9
    trineg = np.where(a >= b, 0.0, -1e9).astype(ml_dtypes.bfloat16)
    return ident, np.ascontiguousarray(trineg)


def kernel(x, Wq, bq, Wk, bk, Wv, bv, Wo, bo, _trace=False, _trace_cores=None):
    if "nc" not in _cache:
        _cache["nc"] = _build()
    nc = _cache["nc"]
    ident, tri = _consts()
    bf = ml_dtypes.bfloat16
    x = np.asarray(x, dtype=np.float32).astype(bf)
    in_maps = []
    for c in range(N_CORES):
        bi, g = divmod(c, TP)
        sl = slice(g * NSL, (g + 1) * NSL)
        in_maps.append({
            "x": np.ascontiguousarray(x[bi]),
            "wq": np.ascontiguousarray(np.asarray(Wq)[:, sl].astype(bf)),
            "bq": np.ascontiguousarray(np.asarray(bq)[sl]),
            "wk": np.ascontiguousarray(np.asarray(Wk)[:, sl].astype(bf)),
            "bk": np.ascontiguousarray(np.asarray(bk)[sl]),
            "wv": np.ascontiguousarray(np.asarray(Wv)[:, sl].astype(bf)),
            "bv": np.ascontiguousarray(np.asarray(bv)[sl]),
            "wo": np.ascontiguousarray(np.asarray(Wo)[:, sl].astype(bf)),
            "bo": np.ascontiguousarray(np.asarray(bo)[sl]),
            "ident": ident,
            "tri": tri,
        })
    res = run_bass_kernel_spmd(
        nc, in_maps, list(range(N_CORES)),
        trace=_trace, trace_cores=_trace_cores)
    out = np.empty((B, S, E), np.float32)
    for c in range(N_CORES):
        bi, g = divmod(c, TP)
        out[bi, :, g * NSL:(g + 1) * NSL] = res.results[c]["out"]
    if _trace:
        _cache["last_result"] = res
    return out

